# revision 5
# baseline (speedup 1.0000x reference)
"""Mixture-of-logistics NLL loss (reduction=mean) on 8 Trainium2 NeuronCores.

Math (per row, K=16 mixture components):
    log_prob = ln(sum_k e^{w_k} pdf_k) - ln(sum_k e^{w_k})
    pdf_k = logistic_pdf(t; loc_k, s_k) = rp_k * sech^2(z_k/2) / 4,
            z_k = (t - loc_k) * rp_k,  rp = 1/s
    sech^2(z/2) = 1 - tanh^2(z/2)
The 1/4 factor is pulled out of the per-row sum and folded into the host
combine as a single -ln(4).

Key design points (v2, from trace analysis of the v1 kernel):
 - v1 was ACT-bound (170us busy of 207us): Ln(s) + Exp(-ln s) for 1/s, plus
   ~15 ACT table loads from rotating ln<->exp table sets every chunk.
 - 1/s is now computed on DVE with a bf16 bit-trick + one Newton step
   (4 cheap DVE ops), so ACT runs only Exp/Tanh/Square -- all in the ONE
   `exp_and_others` table set -> zero steady-state table loads, and ACT work
   drops from 4 to 3 table lookups per element.
 - The two per-row K-sums use a single segmented tensor_reduce(axis=X) each
   instead of a 4-instruction shuffle tree.
 - All elementwise work is in-place over 4 rotating bf16 tiles; inputs are
   cast f32->bf16 in-flight by the SWDGE DMAs (validated ~3e-4 rel error
   including the Newton reciprocal; magic 0x7EF7, bias-trim +0.002).
 - GpSimd only does SWDGE DMA descgen (its tensor ops would lock the SBUF
   port shared with DVE).

Sharding: pure data parallel over rows (batch*seq) across 8 cores; each core
returns per-partition partial sums [128, 2] = (sum ln(num), sum ln(den));
host combines.
"""

import numpy as np

import concourse.bacc as bacc
import concourse.mybir as mybir
import concourse.tile as tile
from concourse.bass_utils import run_bass_kernel_spmd

B, T, K = 16, 131072, 16
N = B * T                 # 2097152 rows total
NCORES = 8
NLOC = N // NCORES        # 262144 rows per core
P = 128                   # SBUF partitions

F32 = mybir.dt.float32
BF16 = mybir.dt.bfloat16
I16 = mybir.dt.int16
AF = mybir.ActivationFunctionType
OP = mybir.AluOpType
AX = mybir.AxisListType

# bf16 reciprocal bit-trick: r0_bits = MAGIC - s_bits, one Newton step
# rp = r0*(2+DELTA - s*r0).  MAGIC/DELTA tuned numerically over the
# s ~ U[0.05, 1) input range: mean rel err ~1e-3, max ~1.1e-2 -> final
# loss error ~3e-4 (bf16 rounding dominated).
MAGIC = 0x7EF7                # r0_bits = MAGIC - s_bits = s_bits*(-1) + MAGIC
DELTA = 0.002


def build_kernel(nloc=NLOC, chunks=None):
    """Build the per-core Bass module."""
    p = P
    r = nloc // p             # rows per partition (2048)
    if chunks is None:
        chunks = [128, 256, 256, 256, 256, 256, 256, 256, 128]
    assert sum(chunks) == r and nloc % p == 0
    cmax = max(chunks)

    nc = bacc.Bacc("TRN2", target_bir_lowering=False, debug=False)
    w_d = nc.dram_tensor("w", [nloc, K], F32, kind="ExternalInput")
    loc_d = nc.dram_tensor("loc", [nloc, K], F32, kind="ExternalInput")
    scale_d = nc.dram_tensor("scale", [nloc, K], F32, kind="ExternalInput")
    t_d = nc.dram_tensor("t", [nloc], F32, kind="ExternalInput")
    out_d = nc.dram_tensor("out", [p, 2], F32, kind="ExternalOutput")

    wv = w_d.ap().rearrange("(p r) k -> p r k", p=p)
    lv = loc_d.ap().rearrange("(p r) k -> p r k", p=p)
    sv = scale_d.ap().rearrange("(p r) k -> p r k", p=p)
    tv = t_d.ap().rearrange("(p r) -> p r", p=p)

    with tile.TileContext(nc) as tc:
        with (
            tc.tile_pool(name="persist", bufs=1) as pp,
            tc.tile_pool(name="psc", bufs=3) as psc,
            tc.tile_pool(name="pwld", bufs=3) as pwld,
            tc.tile_pool(name="plc", bufs=3) as plc,
            tc.tile_pool(name="pr0", bufs=2) as pr0,
            nc.allow_low_precision("bf16 pipeline validated: ~3e-4 rel"),
        ):
            t_all = pp.tile([p, r], BF16)         # targets (bf16)
            stash_n = pp.tile([p, r], F32)        # per-row numerator sums
            stash_d = pp.tile([p, r], F32)        # per-row denominator sums
            out_sb = pp.tile([p, 2], F32)

            nc.gpsimd.dma_start(out=t_all, in_=tv)   # one cast DMA up front

            o = 0
            for c in chunks:
                sl = slice(o, o + c)
                o += c
                sc_t = psc.tile([p, cmax, K], BF16, tag="sc", name="sct")[:, :c, :]
                w_t = pwld.tile([p, cmax, K], BF16, tag="w", name="wt")[:, :c, :]
                loc_t = plc.tile([p, cmax, K], BF16, tag="loc", name="loct")[:, :c, :]
                r0_t = pr0.tile([p, cmax, K], BF16, tag="r0", name="r0t")[:, :c, :]
                # SWDGE DMAs cast f32->bf16 in flight
                nc.gpsimd.dma_start(out=sc_t, in_=sv[:, sl, :])
                nc.gpsimd.dma_start(out=w_t, in_=wv[:, sl, :])
                nc.gpsimd.dma_start(out=loc_t, in_=lv[:, sl, :])

                # ---- rp = 1/s on DVE (bit trick + 1 Newton step) ----
                nc.vector.tensor_scalar(
                    out=r0_t.bitcast(I16), in0=sc_t.bitcast(I16),
                    scalar1=-1, scalar2=MAGIC,
                    op0=OP.mult, op1=OP.add,
                )                                                  # r0
                nc.vector.tensor_mul(out=sc_t, in0=sc_t, in1=r0_t)  # s*r0
                nc.vector.tensor_scalar(
                    out=sc_t, in0=sc_t, scalar1=-1.0, scalar2=2.0 + DELTA,
                    op0=OP.mult, op1=OP.add,
                )                                                  # 2+d-s*r0
                nc.vector.tensor_mul(out=r0_t, in0=r0_t, in1=sc_t)  # rp

                act = nc.scalar.activation
                act(out=w_t, in_=w_t, func=AF.Exp)                  # e^w

                # diff = t - loc (broadcast over K; 1x mode, cheapest option)
                tb = t_all[:, sl].unsqueeze(2).broadcast_to([p, c, K])
                nc.vector.tensor_sub(out=loc_t, in0=tb, in1=loc_t)
                nc.vector.tensor_mul(out=loc_t, in0=loc_t, in1=r0_t)  # v = z
                nc.vector.tensor_mul(out=r0_t, in0=r0_t, in1=w_t)     # pw = rp*e^w

                act(out=loc_t, in_=loc_t, func=AF.Tanh, scale=0.5)    # th
                act(out=loc_t, in_=loc_t, func=AF.Square)             # th^2
                nc.vector.tensor_mul(out=loc_t, in0=loc_t, in1=r0_t)  # th^2*pw
                nc.vector.tensor_sub(out=r0_t, in0=r0_t, in1=loc_t)   # term

                nc.vector.reduce_sum(out=stash_d[:, sl], in_=w_t, axis=AX.X)
                nc.vector.reduce_sum(out=stash_n[:, sl], in_=r0_t, axis=AX.X)

            # ---- per-row logs + per-partition accumulation ----
            nc.scalar.activation(out=stash_n, in_=stash_n, func=AF.Ln,
                                 accum_out=out_sb[:, 0:1])
            nc.scalar.activation(out=stash_d, in_=stash_d, func=AF.Ln,
                                 accum_out=out_sb[:, 1:2])
            nc.gpsimd.dma_start(out=out_d.ap(), in_=out_sb)

    nc.compile()
    return nc


def _combine(outs, n_rows):
    total = 0.0
    for o in outs:
        total += float(o[:, 0].sum(dtype=np.float64))
        total -= float(o[:, 1].sum(dtype=np.float64))
    return np.float32(total / n_rows - np.log(4.0))


def make_in_maps(weight, loc, scale, targets):
    w = np.ascontiguousarray(weight.reshape(N, K), dtype=np.float32)
    l = np.ascontiguousarray(loc.reshape(N, K), dtype=np.float32)
    s = np.ascontiguousarray(scale.reshape(N, K), dtype=np.float32)
    t = np.ascontiguousarray(targets.reshape(N), dtype=np.float32)
    in_maps = []
    for ci in range(NCORES):
        rs = slice(ci * NLOC, (ci + 1) * NLOC)
        in_maps.append({
            "w": np.ascontiguousarray(w[rs]),
            "loc": np.ascontiguousarray(l[rs]),
            "scale": np.ascontiguousarray(s[rs]),
            "t": np.ascontiguousarray(t[rs]),
        })
    return in_maps


def run(in_maps, **kwargs):
    nc = build_kernel()
    return run_bass_kernel_spmd(nc, in_maps, core_ids=list(range(NCORES)), **kwargs)


def kernel(weight, loc, scale, targets):
    in_maps = make_in_maps(weight, loc, scale, targets)
    last = None
    for _ in range(3):  # rare transient NRT device errors: retry
        try:
            res = run(in_maps)
            return _combine([r["out"] for r in res.results], N)
        except Exception as e:  # noqa: BLE001
            last = e
    raise last


if __name__ == "__main__":
    nc = build_kernel()
    print("kernel built OK")


# revision 7
# speedup vs baseline: 1.0800x; 1.0800x over previous
"""Mixture-of-logistics NLL loss (reduction=mean) on 8 Trainium2 NeuronCores.

Math (per row, K=16 mixture components):
    log_prob = ln(sum_k e^{w_k} pdf_k) - ln(sum_k e^{w_k})
    pdf_k = logistic_pdf(t; loc_k, s_k) = rp_k * sech^2(z_k/2) / 4,
            z_k = (t - loc_k) * rp_k,  rp = 1/s
    sech^2(z/2) = 1 - tanh^2(z/2)
The 1/4 factor is pulled out of the per-row sum and folded into the host
combine as a single -ln(4).

Design (v4; evolved via traces of v1/v2/v3):
 - v1 was ACT-bound (170us/207us busy): ln/exp for 1/s + ~15 table loads.
   v2 (DVE Newton bit-trick + tensor_reduce) was DVE-bound (231us/270us):
   tensor_reduce runs at 1x (2x slower than the tree16 shuffle tree) and
   the 4-op Newton costs 7us/chunk of DVE.
 - v4 balance (c=256 chunk: DVE TT@2x=2.3us, TS@4x=1.2us, ACT op=3.7us):
     ACT (one table set, zero steady-state loads):
       tbc = Copy(t broadcast over K)   <- lifts the t-minus-loc broadcast
             subtract off DVE (broadcast tensor_tensor runs at 1x)
       ew = Exp(w);  th = Tanh(z/2);  sq = Square(th)
     DVE:
       rp32 = reciprocal_approx_fast(scale_f32)   (1 custom op, ~51 ULP)
       rpb  = copy rp32 -> bf16                    (2x_2p)
       diff = tbc - loc       (2x, no broadcast)
       v    = diff * rpb      (2x)
       pwn  = (rpb * -1) * ew      [scalar_tensor_tensor] = -rp*e^w
       term = (sq - 1) * pwn       [scalar_tensor_tensor] = +pw*(1-th^2)
       tree16(ew) and tree16(term) row-sums (bf16 shuffle tree, ~2x)
 - scale streams in as raw f32 on the HWDGE (sync) queue; w/loc/t stream
   as f32->bf16 SWDGE cast DMAs on gpsimd (GpSimd does only descgen: its
   tensor ops would lock the SBUF port shared with DVE).
 - Validated end-to-end ~3e-4 rel error vs the fp jax reference.

Sharding: pure data parallel over rows (batch*seq) across 8 cores; each core
returns per-partition partial sums [128, 2] = (sum ln(num), sum ln(den));
host combines (mean - ln 4).
"""

import numpy as np

import concourse.bacc as bacc
import concourse.mybir as mybir
import concourse.tile as tile
from concourse.bass_utils import run_bass_kernel_spmd

B, T, K = 16, 131072, 16
N = B * T                 # 2097152 rows total
NCORES = 8
NLOC = N // NCORES        # 262144 rows per core
P = 128                   # SBUF partitions

F32 = mybir.dt.float32
BF16 = mybir.dt.bfloat16
AF = mybir.ActivationFunctionType
OP = mybir.AluOpType


def build_kernel(nloc=NLOC, chunks=None):
    """Build the per-core Bass module."""
    p = P
    r = nloc // p             # rows per partition (2048)
    if chunks is None:
        chunks = [128, 256, 256, 256, 256, 256, 256, 256, 128]
    assert sum(chunks) == r and nloc % p == 0
    cmax = max(chunks)

    nc = bacc.Bacc("TRN2", target_bir_lowering=False, debug=False)
    w_d = nc.dram_tensor("w", [nloc, K], F32, kind="ExternalInput")
    loc_d = nc.dram_tensor("loc", [nloc, K], F32, kind="ExternalInput")
    scale_d = nc.dram_tensor("scale", [nloc, K], F32, kind="ExternalInput")
    t_d = nc.dram_tensor("t", [nloc], F32, kind="ExternalInput")
    out_d = nc.dram_tensor("out", [p, 2], F32, kind="ExternalOutput")

    wv = w_d.ap().rearrange("(p r) k -> p r k", p=p)
    lv = loc_d.ap().rearrange("(p r) k -> p r k", p=p)
    sv = scale_d.ap().rearrange("(p r) k -> p r k", p=p)
    tv = t_d.ap().rearrange("(p r) -> p r", p=p)

    with tile.TileContext(nc) as tc:
        with (
            tc.tile_pool(name="persist", bufs=1) as pp,
            tc.tile_pool(name="psc", bufs=2) as psc,
            tc.tile_pool(name="prp", bufs=2) as prp,
            tc.tile_pool(name="pwld", bufs=3) as pwld,
            tc.tile_pool(name="plc", bufs=3) as plc,
            tc.tile_pool(name="ptb", bufs=2) as ptb,
            tc.tile_pool(name="pt", bufs=2) as pt,
            nc.allow_low_precision("bf16 pipeline validated: ~3e-4 rel"),
        ):
            t_all = pp.tile([p, r], BF16)         # targets (bf16)
            stash_n = pp.tile([p, r], F32)        # per-row numerator sums
            stash_d = pp.tile([p, r], F32)        # per-row denominator sums
            out_sb = pp.tile([p, 2], F32)

            nc.gpsimd.dma_start(out=t_all, in_=tv)   # one cast DMA up front

            def tree16(src, dst_slice, c):
                """Sum src [p, c, 16] bf16 over last axis -> dst_slice [p, c] f32."""
                t1 = pt.tile([p, cmax, 8], BF16, tag="t1", name="t1")[:, :c, :]
                nc.vector.tensor_add(out=t1, in0=src[:, :, 0:8], in1=src[:, :, 8:16])
                t2 = pt.tile([p, cmax, 4], BF16, tag="t2", name="t2")[:, :c, :]
                nc.vector.tensor_add(out=t2, in0=t1[:, :, 0:4], in1=t1[:, :, 4:8])
                t3 = pt.tile([p, cmax, 2], BF16, tag="t3", name="t3")[:, :c, :]
                nc.vector.tensor_add(out=t3, in0=t2[:, :, 0:2], in1=t2[:, :, 2:4])
                nc.vector.tensor_add(out=dst_slice, in0=t3[:, :, 0], in1=t3[:, :, 1])

            o = 0
            for c in chunks:
                sl = slice(o, o + c)
                o += c
                sc_t = psc.tile([p, cmax, K], F32, tag="sc", name="sct")[:, :c, :]
                rpb = prp.tile([p, cmax, K], BF16, tag="rp", name="rpt")[:, :c, :]
                w_t = pwld.tile([p, cmax, K], BF16, tag="w", name="wt")[:, :c, :]
                loc_t = plc.tile([p, cmax, K], BF16, tag="loc", name="loct")[:, :c, :]
                tbc = ptb.tile([p, cmax, K], BF16, tag="tb", name="tbt")[:, :c, :]

                nc.sync.dma_start(out=sc_t, in_=sv[:, sl, :])     # raw f32, HWDGE
                nc.gpsimd.dma_start(out=w_t, in_=wv[:, sl, :])    # f32->bf16 SWDGE
                nc.gpsimd.dma_start(out=loc_t, in_=lv[:, sl, :])

                # rp = 1/s: one custom-DVE op (~51 ULP), in place over sc
                nc.vector.reciprocal_approx_fast(out=sc_t, in_=sc_t)
                nc.vector.tensor_copy(out=rpb, in_=sc_t)          # f32 -> bf16

                # t broadcast over K on ACT (Copy is in every table set);
                # keeps the t-loc subtract at 2x on DVE.
                tb = t_all[:, sl].unsqueeze(2).broadcast_to([p, c, K])
                nc.scalar.activation(out=tbc, in_=tb, func=AF.Copy)

                nc.scalar.activation(out=w_t, in_=w_t, func=AF.Exp)      # e^w

                nc.vector.tensor_sub(out=loc_t, in0=tbc, in1=loc_t)      # diff
                nc.vector.tensor_mul(out=loc_t, in0=loc_t, in1=rpb)      # z

                nc.scalar.activation(out=loc_t, in_=loc_t, func=AF.Tanh,
                                     scale=0.5)                          # th
                nc.scalar.activation(out=loc_t, in_=loc_t, func=AF.Square)

                tree16(w_t, stash_d[:, sl], c)                    # sum e^w
                # pwn = -rp * e^w ; term = (th^2 - 1) * pwn = +pw*(1-th^2)
                nc.vector.scalar_tensor_tensor(
                    out=rpb, in0=rpb, scalar=-1.0, in1=w_t,
                    op0=OP.mult, op1=OP.mult,
                )
                nc.vector.scalar_tensor_tensor(
                    out=loc_t, in0=loc_t, scalar=1.0, in1=rpb,
                    op0=OP.subtract, op1=OP.mult,
                )
                tree16(loc_t, stash_n[:, sl], c)

            # ---- per-row logs + per-partition accumulation ----
            nc.scalar.activation(out=stash_n, in_=stash_n, func=AF.Ln,
                                 accum_out=out_sb[:, 0:1])
            nc.scalar.activation(out=stash_d, in_=stash_d, func=AF.Ln,
                                 accum_out=out_sb[:, 1:2])
            nc.gpsimd.dma_start(out=out_d.ap(), in_=out_sb)

    nc.compile()
    return nc


def _combine(outs, n_rows):
    total = 0.0
    for o in outs:
        total += float(o[:, 0].sum(dtype=np.float64))
        total -= float(o[:, 1].sum(dtype=np.float64))
    return np.float32(total / n_rows - np.log(4.0))


def make_in_maps(weight, loc, scale, targets):
    w = np.ascontiguousarray(weight.reshape(N, K), dtype=np.float32)
    l = np.ascontiguousarray(loc.reshape(N, K), dtype=np.float32)
    s = np.ascontiguousarray(scale.reshape(N, K), dtype=np.float32)
    t = np.ascontiguousarray(targets.reshape(N), dtype=np.float32)
    in_maps = []
    for ci in range(NCORES):
        rs = slice(ci * NLOC, (ci + 1) * NLOC)
        in_maps.append({
            "w": np.ascontiguousarray(w[rs]),
            "loc": np.ascontiguousarray(l[rs]),
            "scale": np.ascontiguousarray(s[rs]),
            "t": np.ascontiguousarray(t[rs]),
        })
    return in_maps


def run(in_maps, **kwargs):
    nc = build_kernel()
    return run_bass_kernel_spmd(nc, in_maps, core_ids=list(range(NCORES)), **kwargs)


def kernel(weight, loc, scale, targets):
    in_maps = make_in_maps(weight, loc, scale, targets)
    last = None
    for _ in range(3):  # rare transient NRT device errors: retry
        try:
            res = run(in_maps)
            return _combine([r["out"] for r in res.results], N)
        except Exception as e:  # noqa: BLE001
            last = e
    raise last


if __name__ == "__main__":
    nc = build_kernel()
    print("kernel built OK")
